# revision 1
# baseline (speedup 1.0000x reference)
"""Neural CDE (RK4 3/8, 63 steps) Trainium2 Bass kernel — v4.

v1 structure (full-width BL=128, 18 fp32 matmuls/stage, zf muls split
DVE 11 / GPSIMD 5, R broadcast DMA) with the critical-path fixes that
are HW-proven from v3:
  - RK4 arg/h chains read the raw k sums straight from PSUM via DVE
    scalar_tensor_tensor (single kp accumulation group; the ACT k-scale
    round-trip is gone). 10 stt per step, 1 per stage on the critical
    path; the rest execute during the next stage's A-matmul block.
  - The b2 matmul opens the kp group and is issued BEFORE zpre, so the
    PE has work while the crit stt produces the next zpre argument.
  - PSUM tiles padded to a full 2KB bank each (PE-write vs DVE-read on
    the same bank is fatal).
Keeps v1's per-stage idle (~1us front-end) — deliberately NOT fully
pipelined: 100%-duty fp32 PE streams get clamped to 0.6GHz by the
power governor (v3 measured 4.09ms); v1's duty mix oscillates
2.4/1.2GHz. v4 probes the middle (~89% duty).
"""

import numpy as np
import sys

sys.path.insert(0, "/opt/trn_rl_repo")

H, F, B, S = 128, 16, 1024, 64
NC = 8
BL = B // NC          # 128 batch per core
NS = S - 1            # 63 steps
NST = NS * 4          # 252 stages

_compiled = None

# KR product split: f-slices 0..KR_DVE-1 on VectorE, rest on GpSimd
KR_DVE = 11


def _host_prep(x, W1, b1, W2, b2, Wi, bi):
    """Host-side precompute: Hermite-cubic dX table + weight repacks."""
    f32 = np.float32
    x = np.asarray(x, f32)

    times = np.linspace(0.0, 1.0, S, dtype=f32)
    dt = (times[1:] - times[:-1]).astype(np.float64)            # [63]
    dtv = dt[None, :, None]
    xd = x.astype(np.float64)
    p0, p1 = xd[:, :-1], xd[:, 1:]
    seg = (p1 - p0) / dtv                                       # [B, 63, F]
    m0 = np.concatenate([seg[:, :1], seg[:, :-1]], axis=1)
    m1 = seg
    c = (3.0 * seg - (2.0 * m0 + m1)) / dtv
    d = (-2.0 * seg + (m0 + m1)) / (dtv * dtv)
    bco = m0

    # dX(f) = b + (2c + 3d*f)*f at f in {0, dt/3, 2dt/3, dt}
    dX = np.empty((NS, 4, B, F), np.float64)                    # [s, r, b, f]
    for r, frac in enumerate((0.0, 1.0 / 3.0, 2.0 / 3.0, 1.0)):
        fr = (dt * frac)[None, :, None]
        v = bco + (2.0 * c + 3.0 * d * fr) * fr                 # [B, 63, F]
        dX[:, r] = np.swapaxes(v, 0, 1)

    dxt = np.empty((NC, NST, F, BL), f32)
    rflat = np.empty((NC, NST, F * BL), f32)
    for core in range(NC):
        sl = dX[:, :, core * BL:(core + 1) * BL, :]             # [s, r, BL, F]
        t_fb = np.transpose(sl.reshape(NST, BL, F), (0, 2, 1))  # [t, F, BL]
        dxt[core] = t_fb.astype(f32)
        rflat[core] = t_fb.reshape(NST, F * BL).astype(f32)

    W1 = np.asarray(W1, f32); b1 = np.asarray(b1, f32)
    W2 = np.asarray(W2, f32); b2 = np.asarray(b2, f32)
    Wi = np.asarray(Wi, f32); bi = np.asarray(bi, f32)

    w1t = W1.T.astype(f32).copy()                                     # [128, 128]
    apack = np.concatenate(
        [W2[f::F, :].T for f in range(F)], axis=1).astype(f32)        # [128, 2048]
    b2rt = b2.reshape(H, F).T.astype(f32).copy()                      # [16, 128]
    b1c = b1.reshape(H, 1).astype(f32).copy()                         # [128, 1]
    wit = Wi.T.astype(f32).copy()                                     # [16, 128]
    birow = bi.reshape(1, H).astype(f32).copy()                       # [1, 128]
    ones = np.ones((1, BL), f32)

    x0t = np.empty((NC, F, BL), f32)
    for core in range(NC):
        x0t[core] = x[core * BL:(core + 1) * BL, 0, :].T

    in_maps = []
    for core in range(NC):
        in_maps.append({
            "dxt": np.ascontiguousarray(dxt[core]),
            "rflat": np.ascontiguousarray(rflat[core]),
            "w1t": w1t,
            "apack": apack,
            "b2rt": b2rt,
            "b1c": b1c,
            "wit": wit,
            "birow": birow,
            "ones": ones,
            "x0t": np.ascontiguousarray(x0t[core]),
        })
    return in_maps, dt.astype(f32)


def _build(dt_f32):
    """Build + compile the Bass/Tile kernel (shapes and dt are static)."""
    import concourse.bacc as bacc
    import concourse.mybir as mybir
    from concourse import tile

    f32 = mybir.dt.float32
    Tanh = mybir.ActivationFunctionType.Tanh
    Copy = mybir.ActivationFunctionType.Copy
    MUL = mybir.AluOpType.mult
    ADD = mybir.AluOpType.add

    nc = bacc.Bacc("TRN2", target_bir_lowering=False, debug=False)

    d_dxt = nc.dram_tensor("dxt", [NST, F, BL], f32, kind="ExternalInput")
    d_rflat = nc.dram_tensor("rflat", [NST, F * BL], f32, kind="ExternalInput")
    d_w1t = nc.dram_tensor("w1t", [H, H], f32, kind="ExternalInput")
    d_apack = nc.dram_tensor("apack", [H, F * H], f32, kind="ExternalInput")
    d_b2rt = nc.dram_tensor("b2rt", [F, H], f32, kind="ExternalInput")
    d_b1c = nc.dram_tensor("b1c", [H, 1], f32, kind="ExternalInput")
    d_wit = nc.dram_tensor("wit", [F, H], f32, kind="ExternalInput")
    d_birow = nc.dram_tensor("birow", [1, H], f32, kind="ExternalInput")
    d_ones = nc.dram_tensor("ones", [1, BL], f32, kind="ExternalInput")
    d_x0t = nc.dram_tensor("x0t", [F, BL], f32, kind="ExternalInput")
    d_hout = nc.dram_tensor("hout", [H, BL], f32, kind="ExternalOutput")

    with tile.TileContext(nc) as tc:
        with tc.tile_pool(name="const", bufs=1) as cpool, \
             tc.tile_pool(name="work", bufs=2) as wpool, \
             tc.tile_pool(name="rbuf", bufs=3) as rpool, \
             tc.tile_pool(name="dxbuf", bufs=3) as dxpool, \
             tc.tile_pool(name="zfbuf", bufs=2) as zfpool, \
             tc.tile_pool(name="psZ", bufs=2, space="PSUM") as psZ, \
             tc.tile_pool(name="psK", bufs=2, space="PSUM") as psK:

            # ---- load constants to SBUF ----
            sb_w1t = cpool.tile([H, H], f32, tag="w1t")
            sb_apack = cpool.tile([H, F * H], f32, tag="apack")
            sb_b2rt = cpool.tile([F, H], f32, tag="b2rt")
            sb_b1c = cpool.tile([H, 1], f32, tag="b1c")
            sb_wit = cpool.tile([F, H], f32, tag="wit")
            sb_birow = cpool.tile([1, H], f32, tag="birow")
            sb_ones = cpool.tile([1, BL], f32, tag="ones")
            sb_x0t = cpool.tile([F, BL], f32, tag="x0t")
            for sb, dr in ((sb_w1t, d_w1t), (sb_apack, d_apack),
                           (sb_b2rt, d_b2rt), (sb_b1c, d_b1c), (sb_wit, d_wit),
                           (sb_birow, d_birow), (sb_ones, d_ones), (sb_x0t, d_x0t)):
                nc.sync.dma_start(sb[:, :], dr.ap())

            # ---- h0 = Wi @ x0 + bi ----
            p0 = psZ.tile([H, BL], f32, tag="zpre", padded_shape=[H, 512])
            nc.tensor.matmul(p0[:, :], sb_wit[:, :], sb_x0t[:, :],
                             start=True, stop=False)
            nc.tensor.matmul(p0[:, :], sb_birow[:, :], sb_ones[:, :],
                             start=False, stop=True)
            h32 = wpool.tile([H, BL], f32, tag="h32")
            nc.scalar.activation(h32[:, :], p0[:, :], Copy)

            def stt(tag, in0, scalar, in1, name=None):
                out = wpool.tile([H, BL], f32, tag=tag, name=name or tag)
                nc.vector.scalar_tensor_tensor(
                    out[:, :], in0[:, :], float(scalar), in1[:, :],
                    op0=MUL, op1=ADD)
                return out

            st = {"h": h32}
            pending = []       # noncrit stt thunks, emitted after next muls
            for s in range(NS):
                dts = float(dt_f32[s])
                args = [st["h"], None, None, None]
                for i in range(4):
                    t = 4 * s + i
                    # --- per-stage DMAs (prefetched via pool bufs) ---
                    R = rpool.tile([H, F * BL], f32, tag="R")
                    nc.sync.dma_start(
                        R[:, :], d_rflat.ap()[t:t + 1, :].partition_broadcast(H))
                    dxs = dxpool.tile([F, BL], f32, tag="dxs")
                    nc.sync.dma_start(dxs[:, :], d_dxt.ap()[t, :, :])

                    # --- PE: b2 opens kp; runs during the crit-stt hop ---
                    kp = psK.tile([H, BL], f32, tag="kp",
                                  padded_shape=[H, 512])
                    nc.tensor.matmul(kp[:, :], sb_b2rt[:, :], dxs[:, :],
                                     start=True, stop=False)
                    zp = psZ.tile([H, BL], f32, tag="zpre",
                                  padded_shape=[H, 512])
                    nc.tensor.matmul(zp[:, :], sb_w1t[:, :], args[i][:, :],
                                     start=True, stop=True)

                    # --- ACT: zT = tanh(zpre + b1) ---
                    zT = wpool.tile([H, BL], f32, tag="zT")
                    nc.scalar.activation(zT[:, :], zp[:, :], Tanh,
                                         bias=sb_b1c[:, :])

                    # --- KR product: DVE f<KR_DVE, GpSimd rest ---
                    zf = zfpool.tile([H, F * BL], f32, tag="zf")
                    for f in range(F):
                        eng = nc.vector if f < KR_DVE else nc.gpsimd
                        fs = slice(f * BL, (f + 1) * BL)
                        eng.tensor_mul(zf[:, fs], zT[:, :], R[:, fs])

                    # previous stage's noncrit stts execute here, during
                    # this stage's A-matmul block (DVE program order)
                    for thunk in pending:
                        thunk()
                    pending = []

                    # --- PE: A-matmul accumulation into kp ---
                    for f in range(F):
                        fsl = slice(f * H, (f + 1) * H)
                        fb = slice(f * BL, (f + 1) * BL)
                        nc.tensor.matmul(kp[:, :], sb_apack[:, fsl],
                                         zf[:, fb], start=False,
                                         stop=(f == F - 1))

                    # --- DVE: critical chain from PSUM; rest deferred ---
                    h = st["h"]
                    if i == 0:
                        args[1] = stt("a2", kp, dts / 3.0, h)
                        def nc1(kp=kp, h=h):
                            st["t3"] = stt("t3", kp, -dts / 3.0, h)
                            st["t5"] = stt("t5", kp, dts, h)
                            st["u1"] = stt("u1", kp, dts / 8.0, h)
                        pending.append(nc1)
                    elif i == 1:
                        args[2] = stt("a3", kp, dts, st["t3"])
                        def nc2(kp=kp):
                            st["t6"] = stt("t6", kp, -dts, st["t5"])
                            st["u2"] = stt("u2", kp, 3.0 * dts / 8.0,
                                           st["u1"])
                        pending.append(nc2)
                    elif i == 2:
                        args[3] = stt("a4", kp, dts, st["t6"])
                        def nc3(kp=kp):
                            st["u3"] = stt("u3", kp, 3.0 * dts / 8.0,
                                           st["u2"])
                        pending.append(nc3)
                    else:
                        st["h"] = stt("h32", kp, dts / 8.0, st["u3"],
                                      name=f"h32_{s}")

            nc.sync.dma_start(d_hout.ap(), st["h"][:, :])

    nc.compile()
    return nc


def _get_compiled():
    global _compiled
    if _compiled is None:
        f32 = np.float32
        times = np.linspace(0.0, 1.0, S, dtype=f32)
        dt_f32 = times[1:] - times[:-1]
        _compiled = _build(dt_f32)
    return _compiled


def run(inputs, trace=False, trace_kwargs=None):
    """Returns (full_output [B, H] f32, BassKernelResults)."""
    from concourse import bass_utils

    nc = _get_compiled()
    in_maps, _ = _host_prep(**inputs)
    res = bass_utils.run_bass_kernel_spmd(
        nc, in_maps, core_ids=list(range(NC)), trace=trace,
        **(trace_kwargs or {}))
    out = np.concatenate(
        [res.results[c]["hout"].T for c in range(NC)], axis=0)
    return np.ascontiguousarray(out.astype(np.float32)), res


def kernel(**inputs):
    out, _ = run(inputs)
    return out



# revision 2
# speedup vs baseline: 1.0034x; 1.0034x over previous
"""Neural CDE (RK4 3/8, 63 steps) Trainium2 Bass kernel — v4c.

v1 structure (full-width BL=128, 18 fp32 matmuls/stage, zf muls split
DVE 11 / GPSIMD 5, R broadcast DMA) with the critical-path fixes that
are HW-proven from v3:
  - RK4 arg/h chains read the raw k sums straight from PSUM via DVE
    scalar_tensor_tensor (single kp accumulation group; the ACT k-scale
    round-trip is gone). 10 stt per step, 1 per stage on the critical
    path; the rest execute during the next stage's A-matmul block.
  - The b2 matmul opens the kp group and is issued BEFORE zpre, so the
    PE has work while the crit stt produces the next zpre argument.
  - PSUM tiles padded to a full 2KB bank each (PE-write vs DVE-read on
    the same bank is fatal).
Keeps v1's per-stage idle (~1us front-end) — deliberately NOT fully
pipelined: 100%-duty fp32 PE streams get clamped to 0.6GHz by the
power governor (v3 measured 4.09ms); v1's duty mix oscillates
2.4/1.2GHz. v4 probes the middle (~89% duty).
"""

import numpy as np
import sys

sys.path.insert(0, "/opt/trn_rl_repo")

H, F, B, S = 128, 16, 1024, 64
NC = 8
BL = B // NC          # 128 batch per core
NS = S - 1            # 63 steps
NST = NS * 4          # 252 stages

_compiled = None

# KR product split: f-slices 0..KR_DVE-1 on VectorE, rest on GpSimd
KR_DVE = 11


def _host_prep(x, W1, b1, W2, b2, Wi, bi):
    """Host-side precompute: Hermite-cubic dX table + weight repacks."""
    f32 = np.float32
    x = np.asarray(x, f32)

    times = np.linspace(0.0, 1.0, S, dtype=f32)
    dt = (times[1:] - times[:-1]).astype(np.float64)            # [63]
    dtv = dt[None, :, None]
    xd = x.astype(np.float64)
    p0, p1 = xd[:, :-1], xd[:, 1:]
    seg = (p1 - p0) / dtv                                       # [B, 63, F]
    m0 = np.concatenate([seg[:, :1], seg[:, :-1]], axis=1)
    m1 = seg
    c = (3.0 * seg - (2.0 * m0 + m1)) / dtv
    d = (-2.0 * seg + (m0 + m1)) / (dtv * dtv)
    bco = m0

    # dX(f) = b + (2c + 3d*f)*f at f in {0, dt/3, 2dt/3, dt}
    dX = np.empty((NS, 4, B, F), np.float64)                    # [s, r, b, f]
    for r, frac in enumerate((0.0, 1.0 / 3.0, 2.0 / 3.0, 1.0)):
        fr = (dt * frac)[None, :, None]
        v = bco + (2.0 * c + 3.0 * d * fr) * fr                 # [B, 63, F]
        dX[:, r] = np.swapaxes(v, 0, 1)

    dxh = np.empty((NC, NST, F, BL), np.float16)
    dxl = np.empty((NC, NST, F, BL), np.float16)
    rflat = np.empty((NC, NST, F * BL), f32)
    for core in range(NC):
        sl = dX[:, :, core * BL:(core + 1) * BL, :]             # [s, r, BL, F]
        t_fb = np.transpose(sl.reshape(NST, BL, F), (0, 2, 1))  # [t, F, BL]
        hi = t_fb.astype(np.float16)
        dxh[core] = hi
        dxl[core] = (t_fb - hi.astype(np.float64)).astype(np.float16)
        rflat[core] = t_fb.reshape(NST, F * BL).astype(f32)

    W1 = np.asarray(W1, f32); b1 = np.asarray(b1, f32)
    W2 = np.asarray(W2, f32); b2 = np.asarray(b2, f32)
    Wi = np.asarray(Wi, f32); bi = np.asarray(bi, f32)

    w1t = W1.T.astype(f32).copy()                                     # [128, 128]
    apack = np.concatenate(
        [W2[f::F, :].T for f in range(F)], axis=1).astype(f32)        # [128, 2048]
    b2rt = b2.reshape(H, F).T.astype(np.float64)                      # [16, 128]
    b2h = b2rt.astype(np.float16)
    b2l = (b2rt - b2h.astype(np.float64)).astype(np.float16)
    b1c = b1.reshape(H, 1).astype(f32).copy()                         # [128, 1]
    wit = Wi.T.astype(f32).copy()                                     # [16, 128]
    birow = bi.reshape(1, H).astype(f32).copy()                       # [1, 128]
    ones = np.ones((1, BL), f32)

    x0t = np.empty((NC, F, BL), f32)
    for core in range(NC):
        x0t[core] = x[core * BL:(core + 1) * BL, 0, :].T

    in_maps = []
    for core in range(NC):
        in_maps.append({
            "dxh": np.ascontiguousarray(dxh[core]),
            "dxl": np.ascontiguousarray(dxl[core]),
            "rflat": np.ascontiguousarray(rflat[core]),
            "w1t": w1t,
            "apack": apack,
            "b2h": b2h,
            "b2l": b2l,
            "b1c": b1c,
            "wit": wit,
            "birow": birow,
            "ones": ones,
            "x0t": np.ascontiguousarray(x0t[core]),
        })
    return in_maps, dt.astype(f32)


def _build(dt_f32):
    """Build + compile the Bass/Tile kernel (shapes and dt are static)."""
    import concourse.bacc as bacc
    import concourse.mybir as mybir
    from concourse import tile

    f32 = mybir.dt.float32
    Tanh = mybir.ActivationFunctionType.Tanh
    Copy = mybir.ActivationFunctionType.Copy
    MUL = mybir.AluOpType.mult
    ADD = mybir.AluOpType.add

    nc = bacc.Bacc("TRN2", target_bir_lowering=False, debug=False)

    f16 = mybir.dt.float16
    d_dxh = nc.dram_tensor("dxh", [NST, F, BL], f16, kind="ExternalInput")
    d_dxl = nc.dram_tensor("dxl", [NST, F, BL], f16, kind="ExternalInput")
    d_rflat = nc.dram_tensor("rflat", [NST, F * BL], f32, kind="ExternalInput")
    d_w1t = nc.dram_tensor("w1t", [H, H], f32, kind="ExternalInput")
    d_apack = nc.dram_tensor("apack", [H, F * H], f32, kind="ExternalInput")
    d_b2h = nc.dram_tensor("b2h", [F, H], f16, kind="ExternalInput")
    d_b2l = nc.dram_tensor("b2l", [F, H], f16, kind="ExternalInput")
    d_b1c = nc.dram_tensor("b1c", [H, 1], f32, kind="ExternalInput")
    d_wit = nc.dram_tensor("wit", [F, H], f32, kind="ExternalInput")
    d_birow = nc.dram_tensor("birow", [1, H], f32, kind="ExternalInput")
    d_ones = nc.dram_tensor("ones", [1, BL], f32, kind="ExternalInput")
    d_x0t = nc.dram_tensor("x0t", [F, BL], f32, kind="ExternalInput")
    d_hout = nc.dram_tensor("hout", [H, BL], f32, kind="ExternalOutput")

    with tile.TileContext(nc) as tc:
        with tc.tile_pool(name="const", bufs=1) as cpool, \
             tc.tile_pool(name="work", bufs=2) as wpool, \
             tc.tile_pool(name="rbuf", bufs=3) as rpool, \
             tc.tile_pool(name="dxbuf", bufs=3) as dxpool, \
             tc.tile_pool(name="zfbuf", bufs=2) as zfpool, \
             tc.tile_pool(name="psZ", bufs=2, space="PSUM") as psZ, \
             tc.tile_pool(name="psK", bufs=2, space="PSUM") as psK:

            # ---- load constants to SBUF ----
            sb_w1t = cpool.tile([H, H], f32, tag="w1t")
            sb_apack = cpool.tile([H, F * H], f32, tag="apack")
            sb_b2h = cpool.tile([F, H], f16, tag="b2h")
            sb_b2l = cpool.tile([F, H], f16, tag="b2l")
            sb_b1c = cpool.tile([H, 1], f32, tag="b1c")
            sb_wit = cpool.tile([F, H], f32, tag="wit")
            sb_birow = cpool.tile([1, H], f32, tag="birow")
            sb_ones = cpool.tile([1, BL], f32, tag="ones")
            sb_x0t = cpool.tile([F, BL], f32, tag="x0t")
            for sb, dr in ((sb_w1t, d_w1t), (sb_apack, d_apack),
                           (sb_b2h, d_b2h), (sb_b2l, d_b2l),
                           (sb_b1c, d_b1c), (sb_wit, d_wit),
                           (sb_birow, d_birow), (sb_ones, d_ones), (sb_x0t, d_x0t)):
                nc.sync.dma_start(sb[:, :], dr.ap())

            # ---- h0 = Wi @ x0 + bi ----
            p0 = psZ.tile([H, BL], f32, tag="zpre", padded_shape=[H, 512])
            nc.tensor.matmul(p0[:, :], sb_wit[:, :], sb_x0t[:, :],
                             start=True, stop=False)
            nc.tensor.matmul(p0[:, :], sb_birow[:, :], sb_ones[:, :],
                             start=False, stop=True)
            h32 = wpool.tile([H, BL], f32, tag="h32")
            nc.scalar.activation(h32[:, :], p0[:, :], Copy)

            def stt(tag, in0, scalar, in1, name=None):
                out = wpool.tile([H, BL], f32, tag=tag, name=name or tag)
                nc.vector.scalar_tensor_tensor(
                    out[:, :], in0[:, :], float(scalar), in1[:, :],
                    op0=MUL, op1=ADD)
                return out

            st = {"h": h32}
            pending = []       # noncrit stt thunks, emitted after next muls
            for s in range(NS):
                dts = float(dt_f32[s])
                args = [st["h"], None, None, None]
                for i in range(4):
                    t = 4 * s + i
                    # --- per-stage DMAs (prefetched via pool bufs) ---
                    R = rpool.tile([H, F * BL], f32, tag="R")
                    nc.sync.dma_start(
                        R[:, :], d_rflat.ap()[t:t + 1, :].partition_broadcast(H))
                    dxs = dxpool.tile([F, 2 * BL], f16, tag="dxs")
                    nc.sync.dma_start(dxs[:, :BL], d_dxh.ap()[t, :, :])
                    nc.sync.dma_start(dxs[:, BL:], d_dxl.ap()[t, :, :])

                    # --- PE: b2 opens kp; runs during the crit-stt hop ---
                    kp = psK.tile([H, BL], f32, tag="kp",
                                  padded_shape=[H, 512])
                    nc.tensor.matmul(kp[:, :], sb_b2h[:, :], dxs[:, :BL],
                                     start=True, stop=False)
                    nc.tensor.matmul(kp[:, :], sb_b2l[:, :], dxs[:, :BL],
                                     start=False, stop=False)
                    nc.tensor.matmul(kp[:, :], sb_b2h[:, :], dxs[:, BL:],
                                     start=False, stop=False)
                    zp = psZ.tile([H, BL], f32, tag="zpre",
                                  padded_shape=[H, 512])
                    nc.tensor.matmul(zp[:, :], sb_w1t[:, :], args[i][:, :],
                                     start=True, stop=True)

                    # --- ACT: zT = tanh(zpre + b1) ---
                    zT = wpool.tile([H, BL], f32, tag="zT")
                    nc.scalar.activation(zT[:, :], zp[:, :], Tanh,
                                         bias=sb_b1c[:, :])

                    # --- KR product: DVE f<KR_DVE, GpSimd rest.
                    # f0 is on the critical path (gates the first A-MM):
                    # split it into DVE/GP halves so it lands ~2x sooner.
                    zf = zfpool.tile([H, F * BL], f32, tag="zf")
                    hb = BL // 2
                    nc.vector.tensor_mul(zf[:, :hb], zT[:, :hb], R[:, :hb])
                    nc.gpsimd.tensor_mul(zf[:, hb:BL], zT[:, hb:],
                                         R[:, hb:BL])
                    for f in range(1, F):
                        eng = nc.vector if f < KR_DVE else nc.gpsimd
                        fs = slice(f * BL, (f + 1) * BL)
                        eng.tensor_mul(zf[:, fs], zT[:, :], R[:, fs])

                    # previous stage's noncrit stts execute here, during
                    # this stage's A-matmul block (DVE program order)
                    for thunk in pending:
                        thunk()
                    pending = []

                    # --- PE: A-matmul accumulation into kp ---
                    for f in range(F):
                        fsl = slice(f * H, (f + 1) * H)
                        fb = slice(f * BL, (f + 1) * BL)
                        nc.tensor.matmul(kp[:, :], sb_apack[:, fsl],
                                         zf[:, fb], start=False,
                                         stop=(f == F - 1))

                    # --- DVE: critical chain from PSUM; rest deferred ---
                    h = st["h"]
                    if i == 0:
                        args[1] = stt("a2", kp, dts / 3.0, h)
                        def nc1(kp=kp, h=h):
                            st["t3"] = stt("t3", kp, -dts / 3.0, h)
                            st["t5"] = stt("t5", kp, dts, h)
                            st["u1"] = stt("u1", kp, dts / 8.0, h)
                        pending.append(nc1)
                    elif i == 1:
                        args[2] = stt("a3", kp, dts, st["t3"])
                        def nc2(kp=kp):
                            st["t6"] = stt("t6", kp, -dts, st["t5"])
                            st["u2"] = stt("u2", kp, 3.0 * dts / 8.0,
                                           st["u1"])
                        pending.append(nc2)
                    elif i == 2:
                        args[3] = stt("a4", kp, dts, st["t6"])
                        def nc3(kp=kp):
                            st["u3"] = stt("u3", kp, 3.0 * dts / 8.0,
                                           st["u2"])
                        pending.append(nc3)
                    else:
                        st["h"] = stt("h32", kp, dts / 8.0, st["u3"],
                                      name=f"h32_{s}")

            nc.sync.dma_start(d_hout.ap(), st["h"][:, :])

    nc.compile()
    return nc


def _get_compiled():
    global _compiled
    if _compiled is None:
        f32 = np.float32
        times = np.linspace(0.0, 1.0, S, dtype=f32)
        dt_f32 = times[1:] - times[:-1]
        _compiled = _build(dt_f32)
    return _compiled


def run(inputs, trace=False, trace_kwargs=None):
    """Returns (full_output [B, H] f32, BassKernelResults)."""
    from concourse import bass_utils

    nc = _get_compiled()
    in_maps, _ = _host_prep(**inputs)
    res = bass_utils.run_bass_kernel_spmd(
        nc, in_maps, core_ids=list(range(NC)), trace=trace,
        **(trace_kwargs or {}))
    out = np.concatenate(
        [res.results[c]["hout"].T for c in range(NC)], axis=0)
    return np.ascontiguousarray(out.astype(np.float32)), res


def kernel(**inputs):
    out, _ = run(inputs)
    return out

